# revision 8
# baseline (speedup 1.0000x reference)
"""Conditional BatchNorm1d (training mode) on 8 Trainium2 NeuronCores.

Strategy (data-parallel over N):
  - Shard x/labels along N across 8 cores (62500 rows each).
  - One-hot encodings of labels (both layouts) are precomputed host-side in
    bf16 and streamed in (~4 MB/core extra traffic; frees DVE/GPSIMD, whose
    16-partition one-hot builds dominated earlier profiles).
  - Pass 1 (per core): segment sums s1[c,f] = sum_{i: lab=c} x, s2 = sum x^2
    via one-hot matmul on the PE accumulating into PSUM. x is cast to bf16
    during the SWDGE DMA (halves pass-1 HBM traffic; the bf16 rounding error
    cancels statistically in the 31k-sample sums).
  - AllReduce the tiny [16,256] stats across the 8 cores.
  - Stats -> scale/shift [16,256] on-chip (mirrors the reference formulas).
  - Pass 2 (per core): per-row gather of scale/shift via transposed one-hot
    matmul in bf16 with hi/lo split (PSUM accumulation adds the halves, so
    the gather is fp32-exact to ~1e-7), then y = x*s + t on the DVE with
    quad-packed 3-D-AP ops.

Everything is hardcoded for the problem size: x [500000,128] f32,
labels [500000] int, gamma/beta [16,128] f32.
"""
import numpy as np

N_CORES = 8
N = 500000
F = 128
C = 16
EPS = 1e-5

SHARD = N // N_CORES         # 62500 real rows per core
P = 128                      # partitions per tile (16 DMA descriptors/transfer)
J = 20                       # subtiles per group (rows per partition)
GROUP = P * J                # 2560 rows per group
NG = 25                      # groups per core
ROWS = NG * GROUP            # 64000 padded rows per core
QUAD = 4                     # j-subtiles per psum tile / DVE op

_CACHE = {}


def _build():
    import concourse.bacc as bacc
    import concourse.bass as bass
    from concourse import mybir
    import concourse.tile as tile

    F32 = mybir.dt.float32
    BF16 = mybir.dt.bfloat16
    AF = mybir.ActivationFunctionType
    ALU = mybir.AluOpType

    nc = bacc.Bacc("TRN2", target_bir_lowering=False, debug=False,
                   num_devices=N_CORES)
    x = nc.dram_tensor("x", [ROWS, F], F32, kind="ExternalInput").ap()
    xb = nc.dram_tensor("xb", [ROWS, F], BF16, kind="ExternalInput").ap()
    h1 = nc.dram_tensor("h1", [ROWS, C], BF16, kind="ExternalInput").ap()
    ht = nc.dram_tensor("ht", [C, ROWS], BF16, kind="ExternalInput").ap()
    gamma = nc.dram_tensor("gamma", [C, F], F32, kind="ExternalInput").ap()
    beta = nc.dram_tensor("beta", [C, F], F32, kind="ExternalInput").ap()
    invn = nc.dram_tensor("invn", [C, 1], F32, kind="ExternalInput").ap()
    y = nc.dram_tensor("y", [ROWS, F], F32, kind="ExternalOutput").ap()

    with tile.TileContext(nc) as tc:
        with (
            tc.tile_pool(name="const", bufs=1) as const,
            tc.tile_pool(name="small", bufs=1) as small,
            tc.tile_pool(name="dram", bufs=1, space="DRAM") as dram,
            tc.tile_pool(name="psacc", bufs=1, space="PSUM") as psacc,
        ):
            # ---- constants ----
            gamma_sb = const.tile([C, F], F32)
            nc.sync.dma_start(out=gamma_sb[:], in_=gamma)
            beta_sb = const.tile([C, F], F32)
            nc.sync.dma_start(out=beta_sb[:], in_=beta)
            invn_sb = const.tile([C, 1], F32)
            nc.sync.dma_start(out=invn_sb[:], in_=invn)
            eps_sb = const.tile([C, 1], F32)
            nc.vector.memset(eps_sb[:], EPS)

            # ================= PASS 1: local stats =================
            psum_s12 = psacc.tile([C, 2 * F], F32)
            with tc.tile_pool(name="p1", bufs=4) as p1:
                for g in range(NG):
                    base = g * GROUP
                    # p-major: partition p holds rows [base+J*p, base+J*(p+1))
                    x_p = bass.AP(tensor=xb.tensor, offset=base * F,
                                  ap=[[J * F, P], [1, J * F]])
                    # xc = [x (J*F) | x^2 (J*F)]: both halves contiguous;
                    # matmul rhs reads [x_j | xsq_j] via a 2-D free AP.
                    xc = p1.tile([P, 2, J * F], BF16)
                    nc.sync.dma_start(out=xc[:, 0, :].opt(), in_=x_p.opt())
                    nc.scalar.activation(out=xc[:, 1, :].opt(),
                                         in_=xc[:, 0, :].opt(), func=AF.Square)
                    # one-hot H [125, 20, 16] (host-precomputed, contiguous)
                    h_p = bass.AP(tensor=h1.tensor, offset=base * C,
                                  ap=[[J * C, P], [1, J * C]])
                    H = p1.tile([P, J, C], BF16, tag="H")
                    nc.sync.dma_start(out=H[:].opt(), in_=h_p.opt())

                    xc0 = xc[:].opt()
                    for j in range(J):
                        rhs_j = bass.AP(tensor=xc.tensor,
                                        offset=xc0.offset + j * F,
                                        ap=[xc0.ap[0], [J * F, 2], [1, F]])
                        nc.tensor.matmul(
                            out=psum_s12[:],
                            lhsT=H[:, j, :],
                            rhs=rhs_j,
                            start=(g == 0 and j == 0),
                            stop=(g == NG - 1 and j == J - 1),
                        )

            # ================= AllReduce stats =================
            stats_sb = small.tile([C, 2 * F], F32)
            nc.vector.tensor_copy(out=stats_sb[:], in_=psum_s12[:])
            cc_in = dram.tile([C, 2 * F], F32)
            cc_out = dram.tile([C, 2 * F], F32)
            nc.gpsimd.dma_start(out=cc_in[:], in_=stats_sb[:])
            nc.gpsimd.collective_compute(
                "AllReduce",
                mybir.AluOpType.add,
                replica_groups=[list(range(N_CORES))],
                ins=[cc_in.opt()],
                outs=[cc_out.opt()],
            )
            stats_all = small.tile([C, 2 * F], F32)
            nc.gpsimd.dma_start(out=stats_all[:], in_=cc_out[:])

            # ---- stats -> scale/shift (mirrors reference formulas) ----
            mean = small.tile([C, F], F32)
            nc.vector.tensor_scalar(out=mean[:], in0=stats_all[:, 0:F],
                                    scalar1=invn_sb[:], scalar2=None, op0=ALU.mult)
            ex2 = small.tile([C, F], F32)
            nc.vector.tensor_scalar(out=ex2[:], in0=stats_all[:, F:2 * F],
                                    scalar1=invn_sb[:], scalar2=None, op0=ALU.mult)
            var = small.tile([C, F], F32)
            nc.vector.tensor_tensor(out=var[:], in0=mean[:], in1=mean[:], op=ALU.mult)
            nc.vector.tensor_tensor(out=var[:], in0=ex2[:], in1=var[:], op=ALU.subtract)
            std = small.tile([C, F], F32)
            nc.scalar.activation(out=std[:], in_=var[:], func=AF.Sqrt, bias=eps_sb[:])
            istd = small.tile([C, F], F32)
            nc.vector.reciprocal(out=istd[:], in_=std[:])
            sc_sh = small.tile([C, 2 * F], F32)
            nc.vector.tensor_tensor(out=sc_sh[:, 0:F], in0=gamma_sb[:],
                                    in1=istd[:], op=ALU.mult)
            ms = small.tile([C, F], F32)
            nc.vector.tensor_tensor(out=ms[:], in0=mean[:], in1=sc_sh[:, 0:F],
                                    op=ALU.mult)
            nc.vector.tensor_tensor(out=sc_sh[:, F:2 * F], in0=beta_sb[:],
                                    in1=ms[:], op=ALU.subtract)
            # bf16 hi/lo split: hi + lo == sc_sh to ~1e-7 (PSUM adds them)
            sc_hi = small.tile([C, 2 * F], BF16)
            nc.vector.tensor_copy(out=sc_hi[:], in_=sc_sh[:])
            sc_lo = small.tile([C, 2 * F], BF16)
            nc.vector.tensor_tensor(out=sc_lo[:], in0=sc_sh[:], in1=sc_hi[:],
                                    op=ALU.subtract)

            # ================= PASS 2: apply =================
            # p-major x/y; ht columns are host-permuted to (g, j, p) order so
            # lhsT for subtile j is the contiguous slice ht[:, base+125j:...].
            with tc.tile_pool(name="p2", bufs=6) as p2, \
                 tc.tile_pool(name="p2y", bufs=3) as p2y, \
                 tc.tile_pool(name="p2t", bufs=4) as p2t, \
                 tc.tile_pool(name="ps2", bufs=3, space="PSUM") as ps2:
                for g in range(NG):
                    base = g * GROUP
                    x_p = bass.AP(tensor=x.tensor, offset=base * F,
                                  ap=[[J * F, P], [1, J * F]])
                    y_p = bass.AP(tensor=y.tensor, offset=base * F,
                                  ap=[[J * F, P], [1, J * F]])
                    x2_tile = p2.tile([P, J, F], F32)
                    nc.sync.dma_start(out=x2_tile[:].opt(), in_=x_p.opt())
                    ht_ap = bass.AP(tensor=ht.tensor, offset=base,
                                    ap=[[ROWS, C], [1, GROUP]])
                    H_T = p2.tile([C, GROUP], BF16, tag="HT")
                    nc.sync.dma_start(out=H_T[:].opt(), in_=ht_ap.opt())

                    y_tile = p2y.tile([P, J, F], F32)
                    for q in range(J // QUAD):
                        psum_ss = ps2.tile([P, QUAD, 2 * F], F32)  # 2 banks
                        for h in range(QUAD):
                            j = QUAD * q + h
                            lhsT_j = H_T[:, P * j:P * (j + 1)]
                            nc.tensor.matmul(out=psum_ss[:, h, :], lhsT=lhsT_j,
                                             rhs=sc_hi[:], start=True, stop=False)
                            nc.tensor.matmul(out=psum_ss[:, h, :], lhsT=lhsT_j,
                                             rhs=sc_lo[:], start=False, stop=True)
                        j0 = QUAD * q
                        tmp = p2t.tile([P, QUAD, F], F32)
                        nc.vector.tensor_tensor(out=tmp[:],
                                                in0=x2_tile[:, j0:j0 + QUAD, :],
                                                in1=psum_ss[:, :, 0:F],
                                                op=ALU.mult)
                        nc.vector.tensor_tensor(out=y_tile[:, j0:j0 + QUAD, :],
                                                in0=tmp[:],
                                                in1=psum_ss[:, :, F:2 * F],
                                                op=ALU.add)
                    nc.scalar.dma_start(out=y_p.opt(), in_=y_tile[:].opt())
    nc.finalize()
    return nc


def _get_nc():
    if "nc" not in _CACHE:
        _CACHE["nc"] = _build()
    return _CACHE["nc"]


def _prep_host(labels_np):
    import ml_dtypes
    BF = ml_dtypes.bfloat16
    lab = labels_np.astype(np.int64)
    counts = np.maximum(np.bincount(lab, minlength=C), 1).astype(np.float64)
    invn = (1.0 / counts).astype(np.float32).reshape(C, 1)
    eye = np.eye(C, dtype=BF)
    h1_all, ht_all = [], []
    for k in range(N_CORES):
        lab_pad = np.full(ROWS, -1, dtype=np.int64)
        lab_pad[:SHARD] = lab[k * SHARD:(k + 1) * SHARD]
        h1 = np.zeros((ROWS, C), dtype=BF)
        h1[:SHARD] = eye[lab_pad[:SHARD]]
        h1_all.append(h1)
        # ht columns in (g, j, p) order: col g*GROUP+P*j+p holds onehot of
        # padded row g*GROUP + J*p + j (zero for pad rows).
        shard = lab_pad.reshape(NG, P, J)                        # (g, p, j)
        perm = shard.transpose(0, 2, 1).reshape(-1)              # (g, j, p)
        onehot_t = (perm[None, :] == np.arange(C)[:, None])
        ht_all.append(onehot_t.astype(BF))
    return h1_all, ht_all, invn


def kernel(x, labels, gamma, beta):
    from concourse.bass_utils import run_bass_kernel_spmd

    x = np.ascontiguousarray(np.asarray(x, dtype=np.float32))
    labels_np = np.asarray(labels)
    gamma = np.ascontiguousarray(np.asarray(gamma, dtype=np.float32))
    beta = np.ascontiguousarray(np.asarray(beta, dtype=np.float32))

    h1_all, ht_all, invn = _prep_host(labels_np)
    import ml_dtypes
    xb = x.astype(ml_dtypes.bfloat16)

    nc = _get_nc()
    in_maps = []
    for k in range(N_CORES):
        sl = slice(k * SHARD, (k + 1) * SHARD)
        x_pad = np.zeros((ROWS, F), dtype=np.float32)
        x_pad[:SHARD] = x[sl]
        xb_pad = np.zeros((ROWS, F), dtype=ml_dtypes.bfloat16)
        xb_pad[:SHARD] = xb[sl]
        in_maps.append({
            "x": x_pad,
            "xb": xb_pad,
            "h1": h1_all[k],
            "ht": ht_all[k],
            "gamma": gamma,
            "beta": beta,
            "invn": invn,
        })
    res = run_bass_kernel_spmd(nc, in_maps, core_ids=list(range(N_CORES)),
                               **_CACHE.get("run_kwargs", {}))
    _CACHE["last_results"] = res
    y = np.concatenate([res.results[k]["y"][:SHARD] for k in range(N_CORES)],
                       axis=0)
    return y


# revision 9
# speedup vs baseline: 1.1686x; 1.1686x over previous
"""Conditional BatchNorm1d (training mode) on 8 Trainium2 NeuronCores.

Strategy (data-parallel over N):
  - Shard x/labels along N across 8 cores (62500 rows each).
  - One-hot encodings of labels (both layouts) are precomputed host-side in
    bf16 and streamed in (~4 MB/core extra traffic; frees DVE/GPSIMD, whose
    16-partition one-hot builds dominated earlier profiles).
  - Pass 1 (per core): segment sums s1[c,f] = sum_{i: lab=c} x, s2 = sum x^2
    via one-hot matmul on the PE accumulating into PSUM. x is cast to bf16
    during the SWDGE DMA (halves pass-1 HBM traffic; the bf16 rounding error
    cancels statistically in the 31k-sample sums).
  - AllReduce the tiny [16,256] stats across the 8 cores.
  - Stats -> scale/shift [16,256] on-chip (mirrors the reference formulas).
  - Pass 2 (per core): per-row gather of scale/shift via transposed one-hot
    matmul in bf16 with hi/lo split (PSUM accumulation adds the halves, so
    the gather is fp32-exact to ~1e-7), then y = x*s + t on the DVE with
    quad-packed 3-D-AP ops.

Everything is hardcoded for the problem size: x [500000,128] f32,
labels [500000] int, gamma/beta [16,128] f32.
"""
import numpy as np

N_CORES = 8
N = 500000
F = 128
C = 16
EPS = 1e-5

SHARD = N // N_CORES         # 62500 real rows per core
P = 128                      # partitions per tile (16 DMA descriptors/transfer)
J = 20                       # subtiles per group (rows per partition)
GROUP = P * J                # 2560 rows per group
NG = 25                      # groups per core
ROWS = NG * GROUP            # 64000 padded rows per core
QUAD = 4                     # j-subtiles per psum tile / DVE op

_CACHE = {}


def _build():
    import concourse.bacc as bacc
    import concourse.bass as bass
    from concourse import mybir
    import concourse.tile as tile

    F32 = mybir.dt.float32
    BF16 = mybir.dt.bfloat16
    AF = mybir.ActivationFunctionType
    ALU = mybir.AluOpType

    nc = bacc.Bacc("TRN2", target_bir_lowering=False, debug=False,
                   num_devices=N_CORES)
    x = nc.dram_tensor("x", [ROWS, F], F32, kind="ExternalInput").ap()
    xb = nc.dram_tensor("xb", [ROWS, F], BF16, kind="ExternalInput").ap()
    h1 = nc.dram_tensor("h1", [ROWS, C], BF16, kind="ExternalInput").ap()
    ht = nc.dram_tensor("ht", [C, ROWS], BF16, kind="ExternalInput").ap()
    gamma = nc.dram_tensor("gamma", [C, F], F32, kind="ExternalInput").ap()
    beta = nc.dram_tensor("beta", [C, F], F32, kind="ExternalInput").ap()
    invn = nc.dram_tensor("invn", [C, 1], F32, kind="ExternalInput").ap()
    y = nc.dram_tensor("y", [ROWS, F], F32, kind="ExternalOutput").ap()

    with tile.TileContext(nc) as tc:
        with (
            tc.tile_pool(name="const", bufs=1) as const,
            tc.tile_pool(name="small", bufs=1) as small,
            tc.tile_pool(name="dram", bufs=1, space="DRAM") as dram,
            tc.tile_pool(name="psacc", bufs=1, space="PSUM") as psacc,
        ):
            # ---- constants ----
            gamma_sb = const.tile([C, F], F32)
            nc.sync.dma_start(out=gamma_sb[:], in_=gamma)
            beta_sb = const.tile([C, F], F32)
            nc.sync.dma_start(out=beta_sb[:], in_=beta)
            invn_sb = const.tile([C, 1], F32)
            nc.sync.dma_start(out=invn_sb[:], in_=invn)
            eps_sb = const.tile([C, 1], F32)
            nc.vector.memset(eps_sb[:], EPS)

            # ================= PASS 1: local stats =================
            psum_s12 = psacc.tile([C, 2 * F], F32)
            with tc.tile_pool(name="p1", bufs=4) as p1:
                for g in range(NG):
                    base = g * GROUP
                    # p-major: partition p holds rows [base+J*p, base+J*(p+1))
                    x_p = bass.AP(tensor=xb.tensor, offset=base * F,
                                  ap=[[J * F, P], [1, J * F]])
                    # xc = [x (J*F) | x^2 (J*F)]: both halves contiguous;
                    # matmul rhs reads [x_j | xsq_j] via a 2-D free AP.
                    xc = p1.tile([P, 2, J * F], BF16)
                    nc.sync.dma_start(out=xc[:, 0, :].opt(), in_=x_p.opt())
                    nc.scalar.activation(out=xc[:, 1, :].opt(),
                                         in_=xc[:, 0, :].opt(), func=AF.Square)
                    # one-hot H [125, 20, 16] (host-precomputed, contiguous)
                    h_p = bass.AP(tensor=h1.tensor, offset=base * C,
                                  ap=[[J * C, P], [1, J * C]])
                    H = p1.tile([P, J, C], BF16, tag="H")
                    nc.sync.dma_start(out=H[:].opt(), in_=h_p.opt())

                    xc0 = xc[:].opt()
                    for j in range(J):
                        rhs_j = bass.AP(tensor=xc.tensor,
                                        offset=xc0.offset + j * F,
                                        ap=[xc0.ap[0], [J * F, 2], [1, F]])
                        nc.tensor.matmul(
                            out=psum_s12[:],
                            lhsT=H[:, j, :],
                            rhs=rhs_j,
                            start=(g == 0 and j == 0),
                            stop=(g == NG - 1 and j == J - 1),
                        )

            # ================= AllReduce stats =================
            stats_sb = small.tile([C, 2 * F], F32)
            nc.vector.tensor_copy(out=stats_sb[:], in_=psum_s12[:])
            cc_in = dram.tile([C, 2 * F], F32)
            cc_out = dram.tile([C, 2 * F], F32)
            nc.scalar.dma_start(out=cc_in[:], in_=stats_sb[:])
            nc.gpsimd.collective_compute(
                "AllReduce",
                mybir.AluOpType.add,
                replica_groups=[list(range(N_CORES))],
                ins=[cc_in.opt()],
                outs=[cc_out.opt()],
            )
            stats_all = small.tile([C, 2 * F], F32)
            nc.scalar.dma_start(out=stats_all[:], in_=cc_out[:])

            # ---- stats -> scale/shift (mirrors reference formulas) ----
            mean = small.tile([C, F], F32)
            nc.vector.tensor_scalar(out=mean[:], in0=stats_all[:, 0:F],
                                    scalar1=invn_sb[:], scalar2=None, op0=ALU.mult)
            ex2 = small.tile([C, F], F32)
            nc.vector.tensor_scalar(out=ex2[:], in0=stats_all[:, F:2 * F],
                                    scalar1=invn_sb[:], scalar2=None, op0=ALU.mult)
            var = small.tile([C, F], F32)
            nc.vector.tensor_tensor(out=var[:], in0=mean[:], in1=mean[:], op=ALU.mult)
            nc.vector.tensor_tensor(out=var[:], in0=ex2[:], in1=var[:], op=ALU.subtract)
            std = small.tile([C, F], F32)
            nc.scalar.activation(out=std[:], in_=var[:], func=AF.Sqrt, bias=eps_sb[:])
            istd = small.tile([C, F], F32)
            nc.vector.reciprocal(out=istd[:], in_=std[:])
            sc_sh = small.tile([C, 2 * F], F32)
            nc.vector.tensor_tensor(out=sc_sh[:, 0:F], in0=gamma_sb[:],
                                    in1=istd[:], op=ALU.mult)
            ms = small.tile([C, F], F32)
            nc.vector.tensor_tensor(out=ms[:], in0=mean[:], in1=sc_sh[:, 0:F],
                                    op=ALU.mult)
            nc.vector.tensor_tensor(out=sc_sh[:, F:2 * F], in0=beta_sb[:],
                                    in1=ms[:], op=ALU.subtract)
            # bf16 hi/lo split: hi + lo == sc_sh to ~1e-7 (PSUM adds them)
            sc_hi = small.tile([C, 2 * F], BF16)
            nc.vector.tensor_copy(out=sc_hi[:], in_=sc_sh[:])
            sc_lo = small.tile([C, 2 * F], BF16)
            nc.vector.tensor_tensor(out=sc_lo[:], in0=sc_sh[:], in1=sc_hi[:],
                                    op=ALU.subtract)

            # ================= PASS 2: apply =================
            # p-major x/y; ht columns are host-permuted to (g, j, p) order so
            # lhsT for subtile j is the contiguous slice ht[:, base+125j:...].
            with tc.tile_pool(name="p2", bufs=4) as p2, \
                 tc.tile_pool(name="p2y", bufs=3) as p2y, \
                 tc.tile_pool(name="p2t", bufs=4) as p2t, \
                 tc.tile_pool(name="ps2", bufs=3, space="PSUM") as ps2:
                for g in range(NG):
                    base = g * GROUP
                    x_p = bass.AP(tensor=x.tensor, offset=base * F,
                                  ap=[[J * F, P], [1, J * F]])
                    y_p = bass.AP(tensor=y.tensor, offset=base * F,
                                  ap=[[J * F, P], [1, J * F]])
                    x2_tile = p2.tile([P, J, F], F32)
                    nc.sync.dma_start(out=x2_tile[:].opt(), in_=x_p.opt())
                    ht_ap = bass.AP(tensor=ht.tensor, offset=base,
                                    ap=[[ROWS, C], [1, GROUP]])
                    H_T = p2.tile([C, GROUP], BF16, tag="HT")
                    nc.sync.dma_start(out=H_T[:].opt(), in_=ht_ap.opt())

                    y_tile = p2y.tile([P, J, F], F32)
                    for q in range(J // QUAD):
                        psum_ss = ps2.tile([P, QUAD, 2 * F], F32)  # 2 banks
                        for h in range(QUAD):
                            j = QUAD * q + h
                            lhsT_j = H_T[:, P * j:P * (j + 1)]
                            nc.tensor.matmul(out=psum_ss[:, h, :], lhsT=lhsT_j,
                                             rhs=sc_hi[:], start=True, stop=False)
                            nc.tensor.matmul(out=psum_ss[:, h, :], lhsT=lhsT_j,
                                             rhs=sc_lo[:], start=False, stop=True)
                        j0 = QUAD * q
                        tmp = p2t.tile([P, QUAD, F], F32)
                        nc.vector.tensor_tensor(out=tmp[:],
                                                in0=x2_tile[:, j0:j0 + QUAD, :],
                                                in1=psum_ss[:, :, 0:F],
                                                op=ALU.mult)
                        nc.vector.tensor_tensor(out=y_tile[:, j0:j0 + QUAD, :],
                                                in0=tmp[:],
                                                in1=psum_ss[:, :, F:2 * F],
                                                op=ALU.add)
                    nc.scalar.dma_start(out=y_p.opt(), in_=y_tile[:].opt())
    nc.finalize()
    return nc


def _get_nc():
    if "nc" not in _CACHE:
        _CACHE["nc"] = _build()
    return _CACHE["nc"]


def _prep_host(labels_np):
    import ml_dtypes
    BF = ml_dtypes.bfloat16
    lab = labels_np.astype(np.int64)
    counts = np.maximum(np.bincount(lab, minlength=C), 1).astype(np.float64)
    invn = (1.0 / counts).astype(np.float32).reshape(C, 1)
    eye = np.eye(C, dtype=BF)
    h1_all, ht_all = [], []
    for k in range(N_CORES):
        lab_pad = np.full(ROWS, -1, dtype=np.int64)
        lab_pad[:SHARD] = lab[k * SHARD:(k + 1) * SHARD]
        h1 = np.zeros((ROWS, C), dtype=BF)
        h1[:SHARD] = eye[lab_pad[:SHARD]]
        h1_all.append(h1)
        # ht columns in (g, j, p) order: col g*GROUP+P*j+p holds onehot of
        # padded row g*GROUP + J*p + j (zero for pad rows).
        shard = lab_pad.reshape(NG, P, J)                        # (g, p, j)
        perm = shard.transpose(0, 2, 1).reshape(-1)              # (g, j, p)
        onehot_t = (perm[None, :] == np.arange(C)[:, None])
        ht_all.append(onehot_t.astype(BF))
    return h1_all, ht_all, invn


def kernel(x, labels, gamma, beta):
    from concourse.bass_utils import run_bass_kernel_spmd

    x = np.ascontiguousarray(np.asarray(x, dtype=np.float32))
    labels_np = np.asarray(labels)
    gamma = np.ascontiguousarray(np.asarray(gamma, dtype=np.float32))
    beta = np.ascontiguousarray(np.asarray(beta, dtype=np.float32))

    h1_all, ht_all, invn = _prep_host(labels_np)
    import ml_dtypes
    xb = x.astype(ml_dtypes.bfloat16)

    nc = _get_nc()
    in_maps = []
    for k in range(N_CORES):
        sl = slice(k * SHARD, (k + 1) * SHARD)
        x_pad = np.zeros((ROWS, F), dtype=np.float32)
        x_pad[:SHARD] = x[sl]
        xb_pad = np.zeros((ROWS, F), dtype=ml_dtypes.bfloat16)
        xb_pad[:SHARD] = xb[sl]
        in_maps.append({
            "x": x_pad,
            "xb": xb_pad,
            "h1": h1_all[k],
            "ht": ht_all[k],
            "gamma": gamma,
            "beta": beta,
            "invn": invn,
        })
    res = run_bass_kernel_spmd(nc, in_maps, core_ids=list(range(N_CORES)),
                               **_CACHE.get("run_kwargs", {}))
    _CACHE["last_results"] = res
    y = np.concatenate([res.results[k]["y"][:SHARD] for k in range(N_CORES)],
                       axis=0)
    return y
